# revision 16
# baseline (speedup 1.0000x reference)
"""Trainium2 Bass kernel for BasicBlockIMCFlow (quantized ResNet basic block).

Math (all exact integer arithmetic carried in fp32/bf16):
  x_int = rne(x*256)                      (|x*256| << 2^22, clip never binds)
  q1    = clip(floor((x_int+512)/1024), 0, 15)
  h1    = conv3x3(q1, w1); h1 = h1*s1 + b1
  q2    = clip(floor((h1*s1+b1+1024)/2048), 0, 15)
  h2    = conv3x3(q2, w2); h2 = h2*s2 + b2
  out   = clip(h2 + x_int, -32768, 32767) / 256

Floors are computed without fp->int casts via the fp32 "magic number" trick:
  rne(g) = (g + 2^23) - 2^23   for |g| < 2^22
  floor(z) = rne(z - 0.5 + tie_guard)   (z's granularity guarantees no ties)

Convs run on the PE as 9 shifted-tap matmuls over a zero-padded [66,66]
bf16 buffer (activations 0..15 and weights -8..7 are exact in bf16; PSUM
accumulates in fp32, all values < 2^24 so everything is exact).

Data parallel: batch 64 sharded 8 images/core over 8 cores. Two images are
stacked on the 128 SBUF partitions (img0 -> partitions 0:64, img1 -> 64:128);
elementwise stages run full width and each conv tap issues two concurrent
quadrant matmuls (tile_position (0,0) and (64,64)).

Host-side input marshalling: weights are transposed to [in_ch, out_ch, tap],
duplicated onto both partition halves and cast to bf16 (exact: small ints);
the bn affine params are folded into exact dyadic-rational scale/bias pairs.
"""

import os

import numpy as np

_CACHE = {}

B, C, H, W = 64, 64, 64, 64
HW = H * W            # 4096
PW = W + 2            # 66 padded row
N_CORES = 8
IMG_PER_CORE = B // N_CORES   # 8
PAIRS = IMG_PER_CORE // 2     # 4
ROWS_PER_TILE = 8             # output rows per psum tile
NTILES = H // ROWS_PER_TILE   # 8 tiles of [128, 512]
TILE_N = ROWS_PER_TILE * W    # 512

MAGIC = 12582912.0  # 1.5 * 2^23: magic-add sum stays in [2^23, 2^24) where ulp=1


def _build_nc():
    import concourse.bacc as bacc
    import concourse.tile as tile
    import concourse.mybir as mybir
    from contextlib import ExitStack

    f32 = mybir.dt.float32
    bf16 = mybir.dt.bfloat16
    Alu = mybir.AluOpType
    Act = mybir.ActivationFunctionType

    nc = bacc.Bacc()

    x_d = nc.dram_tensor("x", [IMG_PER_CORE, C, HW], f32, kind="ExternalInput")
    w1_d = nc.dram_tensor("w1t", [128, 6 * C], bf16, kind="ExternalInput")
    w2_d = nc.dram_tensor("w2t", [128, 6 * C], bf16, kind="ExternalInput")
    pp_d = nc.dram_tensor("pp", [128, 6], f32, kind="ExternalInput")
    out_d = nc.dram_tensor("out", [IMG_PER_CORE, C, HW], f32, kind="ExternalOutput")

    with tile.TileContext(nc) as tc:
        with ExitStack() as ctx:
            singles = ctx.enter_context(tc.tile_pool(name="singles", bufs=1))
            big = ctx.enter_context(tc.tile_pool(name="big", bufs=2))
            dups = ctx.enter_context(tc.tile_pool(name="dups", bufs=1))
            chunks = ctx.enter_context(tc.tile_pool(name="chunks", bufs=3))
            smalls = ctx.enter_context(tc.tile_pool(name="smalls", bufs=3))
            psum1 = ctx.enter_context(tc.tile_pool(name="psum1", bufs=4, space="PSUM"))
            psum2 = ctx.enter_context(tc.tile_pool(name="psum2", bufs=4, space="PSUM"))

            # weights (host prep): cols [kx*64+o] for kx=0..2 = ky(0,1) pairs
            # stacked on partition halves; cols [192 + kx*64 + o] = ky=2 singles
            # (rows 0:64 valid)
            w1b = singles.tile([128, 6, C], bf16, tag="w1b")
            nc.gpsimd.dma_start(out=w1b, in_=w1_d.rearrange("p (k o) -> p k o", o=C))
            w2b = singles.tile([128, 6, C], bf16, tag="w2b")
            nc.gpsimd.dma_start(out=w2b, in_=w2_d.rearrange("p (k o) -> p k o", o=C))

            # packed per-channel params: cols = [sB, bB, sC, bC]
            pp = singles.tile([128, 6], f32, tag="pp")
            nc.gpsimd.dma_start(out=pp, in_=pp_d[:])
            sB, bB = pp[:, 0:1], pp[:, 1:2]
            sC, bC = pp[:, 2:3], pp[:, 3:4]
            magic_t = pp[:, 4:5]
            neg_magic_t = pp[:, 5:6]

            for p in range(PAIRS):
                i0, i1 = 2 * p, 2 * p + 1

                # ---------- load x pair ----------
                t_t = big.tile([128, HW], f32, tag="t")
                x_pair = x_d[i0:i0 + 2, :, :].rearrange("b c n -> (b c) n")

                qb1 = big.tile([128, H + 2, PW], bf16, tag="qb1")
                qb2 = big.tile([128, H + 2, PW], bf16, tag="qb2")

                # zero the pad borders (interior writes never touch them)
                for qb in (qb1, qb2):
                    nc.vector.memset(qb[:, 0, :], 0.0)
                    nc.vector.memset(qb[:, H + 1, :], 0.0)
                    nc.vector.memset(qb[:, 1:H + 1, 0], 0.0)
                    nc.vector.memset(qb[:, 1:H + 1, PW - 1], 0.0)

                # ---------- stage A (chunked): t, q1, t8 ----------
                NCH = 4
                CW = HW // NCH              # 1024 cols per chunk
                CROWS = H // NCH            # 16 image rows per chunk
                for ch in range(NCH):
                    cs = slice(ch * CW, (ch + 1) * CW)
                    nc.gpsimd.dma_start(out=t_t[:, cs], in_=x_pair[:, cs])
                    # t = rne(x*256) + 2^23   (in-place on the loaded x)
                    nc.scalar.activation(out=t_t[:, cs], in_=t_t[:, cs],
                                         func=Act.Identity, bias=magic_t, scale=256.0)
                    # z = (t - (MAGIC-512)) * 2^-10 = (x_int+512)/1024, exact
                    z = chunks.tile([128, CW], f32, tag="z")
                    nc.vector.tensor_scalar(out=z, in0=t_t[:, cs],
                                            scalar1=MAGIC - 512.0,
                                            scalar2=2.0 ** -10,
                                            op0=Alu.subtract, op1=Alu.mult)
                    # p = (z - 0.49951171875) + MAGIC = MAGIC + floor(z)
                    pm = chunks.tile([128, CW], f32, tag="pm")
                    nc.vector.tensor_scalar(out=pm, in0=z,
                                            scalar1=-0.49951171875,
                                            scalar2=MAGIC, op0=Alu.add,
                                            op1=Alu.add)
                    # e = clamp(p, MAGIC, MAGIC+15) = MAGIC + clip(floor, 0, 15)
                    e = chunks.tile([128, CW], f32, tag="e")
                    nc.vector.tensor_scalar(out=e, in0=pm, scalar1=MAGIC,
                                            scalar2=MAGIC + 15.0, op0=Alu.max,
                                            op1=Alu.min)
                    # q1 = e - MAGIC -> bf16 strided into padded interior (ACT)
                    dst = qb1[:, 1 + ch * CROWS:1 + (ch + 1) * CROWS, 1:W + 1]
                    nc.scalar.activation(out=dst, in_=e, func=Act.Identity,
                                         bias=neg_magic_t, scale=1.0)

                # ---------- conv1 + bn1 + quant2 ----------
                # duplicated per-image buffers: rows 0:64 = image, rows
                # 64:128 = image shifted one padded row (66 elems) down, so a
                # K=128 matmul contracts taps ky and ky+1 at once.
                qda = dups.tile([128, H + 2, PW], bf16, tag="qda")
                qdb = dups.tile([128, H + 2, PW], bf16, tag="qdb")
                nc.gpsimd.dma_start(out=qda[0:64], in_=qb1[0:64])
                nc.gpsimd.dma_start(out=qda[64:128, 0:H + 1, :],
                                    in_=qb1[0:64, 1:H + 2, :])
                nc.gpsimd.dma_start(out=qdb[0:64], in_=qb1[64:128])
                nc.gpsimd.dma_start(out=qdb[64:128, 0:H + 1, :],
                                    in_=qb1[64:128, 1:H + 2, :])
                for j in range(NTILES):
                    ps = psum1.tile([128, TILE_N], f32, tag="ps1")
                    r0 = j * ROWS_PER_TILE
                    for kx in range(3):
                        st = (kx == 0)
                        nc.tensor.matmul(
                            ps[0:64], w1b[:, kx, :],
                            qda[:, r0:r0 + ROWS_PER_TILE, kx:kx + W],
                            start=st, stop=False, tile_position=(0, 0))
                        nc.tensor.matmul(
                            ps[64:128], w1b[:, kx, :],
                            qdb[:, r0:r0 + ROWS_PER_TILE, kx:kx + W],
                            start=st, stop=False, tile_position=(0, 64))
                    for kx in range(3):
                        sp = (kx == 2)
                        nc.tensor.matmul(
                            ps[0:64], w1b[0:64, 3 + kx, :],
                            qda[0:64, r0 + 2:r0 + 2 + ROWS_PER_TILE, kx:kx + W],
                            start=False, stop=sp, tile_position=(0, 0))
                        nc.tensor.matmul(
                            ps[64:128], w1b[0:64, 3 + kx, :],
                            qdb[0:64, r0 + 2:r0 + 2 + ROWS_PER_TILE, kx:kx + W],
                            start=False, stop=sp, tile_position=(0, 64))
                    # g2 = h1*(s1/2048) + (b1+1024)/2048 - 0.5 + 2^-12
                    g2 = smalls.tile([128, TILE_N], f32, tag="g2")
                    nc.scalar.activation(out=g2, in_=ps, func=Act.Identity,
                                         bias=bB, scale=sB)
                    # qr2 = rne(g2) = floor(...)
                    qr2 = smalls.tile([128, TILE_N], f32, tag="qr2")
                    nc.vector.tensor_scalar(out=qr2, in0=g2, scalar1=MAGIC,
                                            scalar2=MAGIC, op0=Alu.add,
                                            op1=Alu.subtract)
                    # q2 = clip -> bf16 strided interior rows
                    dst = qb2[:, 1 + j * ROWS_PER_TILE:1 + (j + 1) * ROWS_PER_TILE,
                              1:W + 1]
                    nc.vector.tensor_scalar(out=dst, in0=qr2, scalar1=0.0,
                                            scalar2=15.0, op0=Alu.max, op1=Alu.min)

                # ---------- conv2 + bn2 + residual + clip ----------
                qdc = dups.tile([128, H + 2, PW], bf16, tag="qdc")
                qdd = dups.tile([128, H + 2, PW], bf16, tag="qdd")
                nc.gpsimd.dma_start(out=qdc[0:64], in_=qb2[0:64])
                nc.gpsimd.dma_start(out=qdc[64:128, 0:H + 1, :],
                                    in_=qb2[0:64, 1:H + 2, :])
                nc.gpsimd.dma_start(out=qdd[0:64], in_=qb2[64:128])
                nc.gpsimd.dma_start(out=qdd[64:128, 0:H + 1, :],
                                    in_=qb2[64:128, 1:H + 2, :])
                for j in range(NTILES):
                    ps = psum2.tile([128, TILE_N], f32, tag="ps2")
                    r0 = j * ROWS_PER_TILE
                    for kx in range(3):
                        st = (kx == 0)
                        nc.tensor.matmul(
                            ps[0:64], w2b[:, kx, :],
                            qdc[:, r0:r0 + ROWS_PER_TILE, kx:kx + W],
                            start=st, stop=False, tile_position=(0, 0))
                        nc.tensor.matmul(
                            ps[64:128], w2b[:, kx, :],
                            qdd[:, r0:r0 + ROWS_PER_TILE, kx:kx + W],
                            start=st, stop=False, tile_position=(0, 64))
                    for kx in range(3):
                        sp = (kx == 2)
                        nc.tensor.matmul(
                            ps[0:64], w2b[0:64, 3 + kx, :],
                            qdc[0:64, r0 + 2:r0 + 2 + ROWS_PER_TILE, kx:kx + W],
                            start=False, stop=sp, tile_position=(0, 0))
                        nc.tensor.matmul(
                            ps[64:128], w2b[0:64, 3 + kx, :],
                            qdd[0:64, r0 + 2:r0 + 2 + ROWS_PER_TILE, kx:kx + W],
                            start=False, stop=sp, tile_position=(0, 64))
                    # u = h2*(s2/256) + b2/256 - 2^15
                    u = smalls.tile([128, TILE_N], f32, tag="u")
                    nc.scalar.activation(out=u, in_=ps, func=Act.Identity,
                                         bias=bC, scale=sC)
                    # v8 = t*2^-8 + u = (h2*s2 + b2 + x_int)/256
                    js = slice(j * TILE_N, (j + 1) * TILE_N)
                    v8 = smalls.tile([128, TILE_N], f32, tag="v8")
                    nc.vector.scalar_tensor_tensor(out=v8, in0=t_t[:, js],
                                                   scalar=2.0 ** -8, in1=u,
                                                   op0=Alu.mult, op1=Alu.add)
                    # out = clip(v8, -128, 127.99609375)
                    ot = smalls.tile([128, TILE_N], f32, tag="ot")
                    nc.vector.tensor_scalar(out=ot, in0=v8, scalar1=-128.0,
                                            scalar2=127.99609375,
                                            op0=Alu.max, op1=Alu.min)
                    nc.gpsimd.dma_start(out=out_d[i0, :, js], in_=ot[0:64])
                    nc.gpsimd.dma_start(out=out_d[i1, :, js], in_=ot[64:128])

    nc.compile()
    return nc


def _get_nc():
    if "nc" not in _CACHE:
        _CACHE["nc"] = _build_nc()
    return _CACHE["nc"]


def _prep_host_inputs(inputs):
    import ml_dtypes

    x = np.ascontiguousarray(inputs["x"], dtype=np.float32).reshape(B, C, HW)

    def wprep(w):
        wt = np.ascontiguousarray(w, dtype=np.float32).reshape(C, C, 3, 3)
        wt = wt.transpose(1, 0, 2, 3)                   # [i, o, ky, kx]
        out = np.zeros((128, 6, C), np.float32)
        for kx in range(3):
            out[0:64, kx, :] = wt[:, :, 0, kx]          # pair rows 0:64 = ky0
            out[64:128, kx, :] = wt[:, :, 1, kx]        # pair rows 64:128 = ky1
            out[0:64, 3 + kx, :] = wt[:, :, 2, kx]      # singles = ky2
        return np.ascontiguousarray(
            out.reshape(128, 6 * C).astype(ml_dtypes.bfloat16))

    w1t = wprep(inputs["w1"])
    w2t = wprep(inputs["w2"])

    s1 = np.asarray(inputs["bn1_scale"], dtype=np.float64)
    b1 = np.asarray(inputs["bn1_bias"], dtype=np.float64)
    s2 = np.asarray(inputs["bn2_scale"], dtype=np.float64)
    b2 = np.asarray(inputs["bn2_bias"], dtype=np.float64)
    # all exact dyadic rationals -> float32 conversion is exact
    sB = (s1 * 2.0 ** -11).astype(np.float32)
    bB = (b1 * 2.0 ** -11 + 2.0 ** -12).astype(np.float32)
    sC = (s2 * 2.0 ** -8).astype(np.float32)
    bC = (b2 * 2.0 ** -8 - 49152.0).astype(np.float32)
    mg = np.full(64, MAGIC, dtype=np.float32)
    pp = np.stack([sB, bB, sC, bC, mg, -mg], axis=1)    # [64, 6]
    pp = np.ascontiguousarray(np.concatenate([pp, pp], axis=0))  # [128, 5]

    return x, w1t, w2t, pp


def kernel(**inputs):
    from concourse.bass_utils import run_bass_kernel_spmd

    x, w1t, w2t, pp = _prep_host_inputs(inputs)
    nc = _get_nc()
    in_maps = []
    for i in range(N_CORES):
        shard = np.ascontiguousarray(x[i * IMG_PER_CORE:(i + 1) * IMG_PER_CORE])
        in_maps.append({"x": shard, "w1t": w1t, "w2t": w2t, "pp": pp})

    trace = bool(int(os.environ.get("KERNEL_TRACE", "0")))
    res = run_bass_kernel_spmd(nc, in_maps, core_ids=list(range(N_CORES)),
                               trace=trace)
    _CACHE["last_results"] = res
    out = np.concatenate([r["out"] for r in res.results], axis=0)
    return out.reshape(B, C, H, W).astype(np.float32)


# revision 17
# speedup vs baseline: 1.5195x; 1.5195x over previous
"""Trainium2 Bass kernel for BasicBlockIMCFlow (quantized ResNet basic block).

Math (all exact integer arithmetic carried in fp32/bf16):
  x_int = rne(x*256)                      (|x*256| << 2^22, clip never binds)
  q1    = clip(floor((x_int+512)/1024), 0, 15)
  h1    = conv3x3(q1, w1); h1 = h1*s1 + b1
  q2    = clip(floor((h1*s1+b1+1024)/2048), 0, 15)
  h2    = conv3x3(q2, w2); h2 = h2*s2 + b2
  out   = clip(h2 + x_int, -32768, 32767) / 256

Floors are computed without fp->int casts via the fp32 "magic number" trick:
  rne(g) = (g + 2^23) - 2^23   for |g| < 2^22
  floor(z) = rne(z - 0.5 + tie_guard)   (z's granularity guarantees no ties)

Convs run on the PE as 9 shifted-tap matmuls over a zero-padded [66,66]
bf16 buffer (activations 0..15 and weights -8..7 are exact in bf16; PSUM
accumulates in fp32, all values < 2^24 so everything is exact).

Data parallel: batch 64 sharded 8 images/core over 8 cores. Two images are
stacked on the 128 SBUF partitions (img0 -> partitions 0:64, img1 -> 64:128);
elementwise stages run full width and each conv tap issues two concurrent
quadrant matmuls (tile_position (0,0) and (64,64)).

Host-side input marshalling: weights are transposed to [in_ch, out_ch, tap],
duplicated onto both partition halves and cast to bf16 (exact: small ints);
the bn affine params are folded into exact dyadic-rational scale/bias pairs.
"""

import os

import numpy as np

_CACHE = {}

B, C, H, W = 64, 64, 64, 64
HW = H * W            # 4096
PW = W + 2            # 66 padded row
N_CORES = 8
IMG_PER_CORE = B // N_CORES   # 8
PAIRS = IMG_PER_CORE // 2     # 4
ROWS_PER_TILE = 8             # output rows per psum tile
NTILES = H // ROWS_PER_TILE   # 8 tiles of [128, 512]
TILE_N = ROWS_PER_TILE * W    # 512

MAGIC = 12582912.0  # 1.5 * 2^23: magic-add sum stays in [2^23, 2^24) where ulp=1


def _build_nc():
    import concourse.bacc as bacc
    import concourse.tile as tile
    import concourse.mybir as mybir
    from contextlib import ExitStack

    f32 = mybir.dt.float32
    bf16 = mybir.dt.bfloat16
    fp8 = mybir.dt.float8e4
    Alu = mybir.AluOpType
    Act = mybir.ActivationFunctionType

    nc = bacc.Bacc()

    x_d = nc.dram_tensor("x", [IMG_PER_CORE, C, HW], f32, kind="ExternalInput")
    w1_d = nc.dram_tensor("w1t", [128, 6 * C], fp8, kind="ExternalInput")
    w2_d = nc.dram_tensor("w2t", [128, 6 * C], fp8, kind="ExternalInput")
    pp_d = nc.dram_tensor("pp", [128, 6], f32, kind="ExternalInput")
    out_d = nc.dram_tensor("out", [IMG_PER_CORE, C, HW], f32, kind="ExternalOutput")

    with tile.TileContext(nc) as tc:
        with ExitStack() as ctx:
            singles = ctx.enter_context(tc.tile_pool(name="singles", bufs=1))
            big = ctx.enter_context(tc.tile_pool(name="big", bufs=2))
            dups = ctx.enter_context(tc.tile_pool(name="dups", bufs=2))
            chunks = ctx.enter_context(tc.tile_pool(name="chunks", bufs=3))
            smalls = ctx.enter_context(tc.tile_pool(name="smalls", bufs=3))
            psum1 = ctx.enter_context(tc.tile_pool(name="psum1", bufs=4, space="PSUM"))
            psum2 = ctx.enter_context(tc.tile_pool(name="psum2", bufs=4, space="PSUM"))

            # weights (host prep): cols [kx*64+o] for kx=0..2 = ky(0,1) pairs
            # stacked on partition halves; cols [192 + kx*64 + o] = ky=2 singles
            # (rows 0:64 valid)
            w1b = singles.tile([128, 6, C], fp8, tag="w1b")
            nc.gpsimd.dma_start(out=w1b, in_=w1_d.rearrange("p (k o) -> p k o", o=C))
            w2b = singles.tile([128, 6, C], fp8, tag="w2b")
            nc.gpsimd.dma_start(out=w2b, in_=w2_d.rearrange("p (k o) -> p k o", o=C))

            # packed per-channel params: cols = [sB, bB, sC, bC]
            pp = singles.tile([128, 6], f32, tag="pp")
            nc.gpsimd.dma_start(out=pp, in_=pp_d[:])
            sB, bB = pp[:, 0:1], pp[:, 1:2]
            sC, bC = pp[:, 2:3], pp[:, 3:4]
            magic_t = pp[:, 4:5]
            neg_magic_t = pp[:, 5:6]

            for p in range(PAIRS):
                i0, i1 = 2 * p, 2 * p + 1

                # ---------- load x pair ----------
                t_t = big.tile([128, HW], f32, tag="t")
                x_pair = x_d[i0:i0 + 2, :, :].rearrange("b c n -> (b c) n")

                qb1 = big.tile([128, H + 2, PW], fp8, tag="qb1")
                qb2 = big.tile([128, H + 2, PW], fp8, tag="qb2")

                # zero the pad borders (interior writes never touch them)
                for qb in (qb1, qb2):
                    nc.vector.memset(qb[:, 0, :], 0.0)
                    nc.vector.memset(qb[:, H + 1, :], 0.0)
                    nc.vector.memset(qb[:, 1:H + 1, 0], 0.0)
                    nc.vector.memset(qb[:, 1:H + 1, PW - 1], 0.0)

                # ---------- stage A (chunked): t, q1, t8 ----------
                NCH = 4
                CW = HW // NCH              # 1024 cols per chunk
                CROWS = H // NCH            # 16 image rows per chunk
                for ch in range(NCH):
                    cs = slice(ch * CW, (ch + 1) * CW)
                    nc.gpsimd.dma_start(out=t_t[:, cs], in_=x_pair[:, cs])
                    # t = rne(x*256) + 2^23   (in-place on the loaded x)
                    nc.scalar.activation(out=t_t[:, cs], in_=t_t[:, cs],
                                         func=Act.Identity, bias=magic_t, scale=256.0)
                    # z = (t - (MAGIC-512)) * 2^-10 = (x_int+512)/1024, exact
                    z = chunks.tile([128, CW], f32, tag="z")
                    nc.vector.tensor_scalar(out=z, in0=t_t[:, cs],
                                            scalar1=MAGIC - 512.0,
                                            scalar2=2.0 ** -10,
                                            op0=Alu.subtract, op1=Alu.mult)
                    # p = (z - 0.49951171875) + MAGIC = MAGIC + floor(z)
                    pm = chunks.tile([128, CW], f32, tag="pm")
                    nc.vector.tensor_scalar(out=pm, in0=z,
                                            scalar1=-0.49951171875,
                                            scalar2=MAGIC, op0=Alu.add,
                                            op1=Alu.add)
                    # e = clamp(p, MAGIC, MAGIC+15) = MAGIC + clip(floor, 0, 15)
                    e = chunks.tile([128, CW], f32, tag="e")
                    nc.vector.tensor_scalar(out=e, in0=pm, scalar1=MAGIC,
                                            scalar2=MAGIC + 15.0, op0=Alu.max,
                                            op1=Alu.min)
                    # q1 = e - MAGIC -> bf16 strided into padded interior (ACT)
                    dst = qb1[:, 1 + ch * CROWS:1 + (ch + 1) * CROWS, 1:W + 1]
                    nc.scalar.activation(out=dst, in_=e, func=Act.Identity,
                                         bias=neg_magic_t, scale=1.0)

                # ---------- conv1 + bn1 + quant2 ----------
                # duplicated per-image buffers: rows 0:64 = image, rows
                # 64:128 = image shifted one padded row (66 elems) down, so a
                # K=128 matmul contracts taps ky and ky+1 at once.
                qda = dups.tile([128, H + 2, PW], fp8, tag="qda")
                qdb = dups.tile([128, H + 2, PW], fp8, tag="qdb")
                nc.gpsimd.dma_start(out=qda[0:64], in_=qb1[0:64])
                nc.gpsimd.dma_start(out=qda[64:128, 0:H + 1, :],
                                    in_=qb1[0:64, 1:H + 2, :])
                nc.gpsimd.dma_start(out=qdb[0:64], in_=qb1[64:128])
                nc.gpsimd.dma_start(out=qdb[64:128, 0:H + 1, :],
                                    in_=qb1[64:128, 1:H + 2, :])
                nc.vector.memset(qda[64:128, H + 1, :], 0.0)
                nc.vector.memset(qdb[64:128, H + 1, :], 0.0)
                for j in range(NTILES):
                    ps = psum1.tile([128, TILE_N], f32, tag="ps1")
                    r0 = j * ROWS_PER_TILE
                    for kx in range(3):
                        st = (kx == 0)
                        nc.tensor.matmul(
                            ps[0:64], w1b[:, kx, :],
                            qda[:, r0:r0 + ROWS_PER_TILE, kx:kx + W],
                            start=st, stop=False, tile_position=(0, 0))
                        nc.tensor.matmul(
                            ps[64:128], w1b[:, kx, :],
                            qdb[:, r0:r0 + ROWS_PER_TILE, kx:kx + W],
                            start=st, stop=False, tile_position=(0, 64))
                    for kx in range(3):
                        sp = (kx == 2)
                        nc.tensor.matmul(
                            ps[0:64], w1b[:, 3 + kx, :],
                            qda[:, r0 + 2:r0 + 2 + ROWS_PER_TILE, kx:kx + W],
                            start=False, stop=sp, tile_position=(0, 0))
                        nc.tensor.matmul(
                            ps[64:128], w1b[:, 3 + kx, :],
                            qdb[:, r0 + 2:r0 + 2 + ROWS_PER_TILE, kx:kx + W],
                            start=False, stop=sp, tile_position=(0, 64))
                    # g2 = h1*(s1/2048) + (b1+1024)/2048 - 0.5 + 2^-12
                    g2 = smalls.tile([128, TILE_N], f32, tag="g2")
                    nc.scalar.activation(out=g2, in_=ps, func=Act.Identity,
                                         bias=bB, scale=sB)
                    # qr2 = rne(g2) = floor(...)
                    qr2 = smalls.tile([128, TILE_N], f32, tag="qr2")
                    nc.vector.tensor_scalar(out=qr2, in0=g2, scalar1=MAGIC,
                                            scalar2=MAGIC, op0=Alu.add,
                                            op1=Alu.subtract)
                    # q2 = clip -> bf16 strided interior rows
                    dst = qb2[:, 1 + j * ROWS_PER_TILE:1 + (j + 1) * ROWS_PER_TILE,
                              1:W + 1]
                    nc.vector.tensor_scalar(out=dst, in0=qr2, scalar1=0.0,
                                            scalar2=15.0, op0=Alu.max, op1=Alu.min)

                # ---------- conv2 + bn2 + residual + clip ----------
                qdc = dups.tile([128, H + 2, PW], fp8, tag="qdc")
                qdd = dups.tile([128, H + 2, PW], fp8, tag="qdd")
                nc.gpsimd.dma_start(out=qdc[0:64], in_=qb2[0:64])
                nc.gpsimd.dma_start(out=qdc[64:128, 0:H + 1, :],
                                    in_=qb2[0:64, 1:H + 2, :])
                nc.gpsimd.dma_start(out=qdd[0:64], in_=qb2[64:128])
                nc.gpsimd.dma_start(out=qdd[64:128, 0:H + 1, :],
                                    in_=qb2[64:128, 1:H + 2, :])
                nc.vector.memset(qdc[64:128, H + 1, :], 0.0)
                nc.vector.memset(qdd[64:128, H + 1, :], 0.0)
                for j in range(NTILES):
                    ps = psum2.tile([128, TILE_N], f32, tag="ps2")
                    r0 = j * ROWS_PER_TILE
                    for kx in range(3):
                        st = (kx == 0)
                        nc.tensor.matmul(
                            ps[0:64], w2b[:, kx, :],
                            qdc[:, r0:r0 + ROWS_PER_TILE, kx:kx + W],
                            start=st, stop=False, tile_position=(0, 0))
                        nc.tensor.matmul(
                            ps[64:128], w2b[:, kx, :],
                            qdd[:, r0:r0 + ROWS_PER_TILE, kx:kx + W],
                            start=st, stop=False, tile_position=(0, 64))
                    for kx in range(3):
                        sp = (kx == 2)
                        nc.tensor.matmul(
                            ps[0:64], w2b[:, 3 + kx, :],
                            qdc[:, r0 + 2:r0 + 2 + ROWS_PER_TILE, kx:kx + W],
                            start=False, stop=sp, tile_position=(0, 0))
                        nc.tensor.matmul(
                            ps[64:128], w2b[:, 3 + kx, :],
                            qdd[:, r0 + 2:r0 + 2 + ROWS_PER_TILE, kx:kx + W],
                            start=False, stop=sp, tile_position=(0, 64))
                    # u = h2*(s2/256) + b2/256 - 2^15
                    u = smalls.tile([128, TILE_N], f32, tag="u")
                    nc.scalar.activation(out=u, in_=ps, func=Act.Identity,
                                         bias=bC, scale=sC)
                    # v8 = t*2^-8 + u = (h2*s2 + b2 + x_int)/256
                    js = slice(j * TILE_N, (j + 1) * TILE_N)
                    v8 = smalls.tile([128, TILE_N], f32, tag="v8")
                    nc.vector.scalar_tensor_tensor(out=v8, in0=t_t[:, js],
                                                   scalar=2.0 ** -8, in1=u,
                                                   op0=Alu.mult, op1=Alu.add)
                    # out = clip(v8, -128, 127.99609375)
                    ot = smalls.tile([128, TILE_N], f32, tag="ot")
                    nc.vector.tensor_scalar(out=ot, in0=v8, scalar1=-128.0,
                                            scalar2=127.99609375,
                                            op0=Alu.max, op1=Alu.min)
                    nc.gpsimd.dma_start(out=out_d[i0, :, js], in_=ot[0:64])
                    nc.gpsimd.dma_start(out=out_d[i1, :, js], in_=ot[64:128])

    nc.compile()
    return nc


def _get_nc():
    if "nc" not in _CACHE:
        _CACHE["nc"] = _build_nc()
    return _CACHE["nc"]


def _prep_host_inputs(inputs):
    import ml_dtypes

    x = np.ascontiguousarray(inputs["x"], dtype=np.float32).reshape(B, C, HW)

    def wprep(w):
        wt = np.ascontiguousarray(w, dtype=np.float32).reshape(C, C, 3, 3)
        wt = wt.transpose(1, 0, 2, 3)                   # [i, o, ky, kx]
        out = np.zeros((128, 6, C), np.float32)
        for kx in range(3):
            out[0:64, kx, :] = wt[:, :, 0, kx]          # pair rows 0:64 = ky0
            out[64:128, kx, :] = wt[:, :, 1, kx]        # pair rows 64:128 = ky1
            out[0:64, 3 + kx, :] = wt[:, :, 2, kx]      # singles = ky2
        import concourse.mybir as mybir
        fp8np = mybir.dt.np(mybir.dt.float8e4)
        return np.ascontiguousarray(
            out.reshape(128, 6 * C).astype(fp8np))

    w1t = wprep(inputs["w1"])
    w2t = wprep(inputs["w2"])

    s1 = np.asarray(inputs["bn1_scale"], dtype=np.float64)
    b1 = np.asarray(inputs["bn1_bias"], dtype=np.float64)
    s2 = np.asarray(inputs["bn2_scale"], dtype=np.float64)
    b2 = np.asarray(inputs["bn2_bias"], dtype=np.float64)
    # all exact dyadic rationals -> float32 conversion is exact
    sB = (s1 * 2.0 ** -11).astype(np.float32)
    bB = (b1 * 2.0 ** -11 + 2.0 ** -12).astype(np.float32)
    sC = (s2 * 2.0 ** -8).astype(np.float32)
    bC = (b2 * 2.0 ** -8 - 49152.0).astype(np.float32)
    mg = np.full(64, MAGIC, dtype=np.float32)
    pp = np.stack([sB, bB, sC, bC, mg, -mg], axis=1)    # [64, 6]
    pp = np.ascontiguousarray(np.concatenate([pp, pp], axis=0))  # [128, 5]

    return x, w1t, w2t, pp


def kernel(**inputs):
    from concourse.bass_utils import run_bass_kernel_spmd

    x, w1t, w2t, pp = _prep_host_inputs(inputs)
    nc = _get_nc()
    in_maps = []
    for i in range(N_CORES):
        shard = np.ascontiguousarray(x[i * IMG_PER_CORE:(i + 1) * IMG_PER_CORE])
        in_maps.append({"x": shard, "w1t": w1t, "w2t": w2t, "pp": pp})

    trace = bool(int(os.environ.get("KERNEL_TRACE", "0")))
    res = run_bass_kernel_spmd(nc, in_maps, core_ids=list(range(N_CORES)),
                               trace=trace)
    _CACHE["last_results"] = res
    out = np.concatenate([r["out"] for r in res.results], axis=0)
    return out.reshape(B, C, H, W).astype(np.float32)
